# revision 10
# baseline (speedup 1.0000x reference)
"""Multi-head attention (B=8, N=1024, D=768, H=12) on 8 TRN2 NeuronCores.

Sharding: pure data parallel over batch — each core handles one batch
element; weights are replicated. No collectives.

Per-core kernel:
  1. qk^T [1536, 1024] = w_qk^T @ x^T in f32r (x fed pre-transposed
     from host); bias folded in as a K=1 matmul.
  2. per head h: scores^T [k, q] = k_h @ q_h^T — K=64, two heads packed
     onto PE row groups 0-63 / 64-127.
  3. softmax without max-subtraction (scores ~ N(0,1): exp overflow
     impossible): attnT = exp(scoresT * scale) from PSUM on ACT, bf16.
  4. attn@v in bf16: psum[0:65] = [v_h | ones]^T @ attnT — the ones
     column yields the softmax denominator in row 64 of the same
     PE stream.
  5. normalize: recip(den) via SBUF, K=1 broadcast matmul, multiply.
  6. proj: y = attn_out^T^T @ w_proj + bias (K=1 matmul), DMA out.

Head-pair rounds are software-pipelined: round p computes scores+exp
for pair p while the PE consumes pair p-1's attnT tiles (exp'd last
round) for attn@v — the PE never waits on ACT mid-round, keeping the
HAM clock gate warm.
"""

import sys

sys.path.insert(0, "/opt/trn_rl_repo")

import numpy as np

B, N, D, H, HD = 8, 1024, 768, 12, 64
F_QK = 2 * D  # 1536
SCALE = HD**-0.5
TOK_TILES = N // 128  # 8
D_SUB = D // 128  # 6
N_CORES = 8

_cached_nc = None


def _build():
    import concourse.tile as tile
    from concourse import bacc, mybir

    F32 = mybir.dt.float32
    F32R = mybir.dt.float32r
    BF16 = mybir.dt.bfloat16
    EXP = mybir.ActivationFunctionType.Exp
    MULT = mybir.AluOpType.mult

    nc = bacc.Bacc("TRN2", target_bir_lowering=False, debug=False)

    xt_d = nc.dram_tensor("xt", [D, N], F32R, kind="ExternalInput").ap()
    wqkv_d = nc.dram_tensor("wqkv", [D, 3 * D], F32R, kind="ExternalInput").ap()
    bqkv_d = nc.dram_tensor("bqkv", [3 * D], F32R, kind="ExternalInput").ap()
    wproj_d = nc.dram_tensor("wproj", [D, D], F32R, kind="ExternalInput").ap()
    bproj_d = nc.dram_tensor("bproj", [D], F32R, kind="ExternalInput").ap()
    y_d = nc.dram_tensor("y", [N, D], F32, kind="ExternalOutput").ap()

    with tile.TileContext(nc) as tc:
        with (
            tc.tile_pool(name="singles", bufs=1) as singles,
            tc.tile_pool(name="qkT", bufs=5) as qkT_pool,
            tc.tile_pool(name="wqk", bufs=3) as wqk_pool,
            tc.tile_pool(name="attnT", bufs=18) as attnT_pool,
            tc.tile_pool(name="den", bufs=1) as den_pool,
            tc.tile_pool(name="yout", bufs=3) as y_pool,
            tc.tile_pool(name="ps", bufs=4, space="PSUM") as ps,
        ):
            # ---- resident SBUF tensors ----
            xT_sb = singles.tile([128, D_SUB, N], F32R)  # 24KB/part
            v_sb = singles.tile([128, TOK_TILES, H * 65], BF16)  # 12.2KB/part
            aoT_sb = singles.tile([128, D_SUB, N], F32R)  # 24KB/part
            wproj_sb = singles.tile([128, D_SUB, D], F32R)  # 18KB/part
            wv_sb = singles.tile([128, D_SUB, D], F32R)  # 18KB/part
            bqk_sb = singles.tile([1, F_QK], F32R)
            bv_sb = singles.tile([1, D], F32R)
            bp_sb = singles.tile([1, D], F32R)
            ones1 = singles.tile([1, 512], F32R)
            ones64 = singles.tile([128, 64], F32R)
            onesb16 = singles.tile([128, 96], BF16)
            ones_f = singles.tile([128, 512], F32)

            # ---- setup ----
            xt_r = xt_d.rearrange("(o p) n -> p o n", p=128)
            for d in range(D_SUB):
                nc.sync.dma_start(xT_sb[:, d, :], xt_r[:, d, :])
            nc.sync.dma_start(
                wv_sb, wqkv_d[:, F_QK:].rearrange("(o p) f -> p o f", p=128)
            )
            nc.sync.dma_start(wproj_sb, wproj_d.rearrange("(o p) f -> p o f", p=128))
            nc.sync.dma_start(bqk_sb, bqkv_d[None, 0:F_QK])
            nc.sync.dma_start(bv_sb, bqkv_d[None, F_QK : 3 * D])
            nc.sync.dma_start(bp_sb, bproj_d[None, :])
            nc.vector.memset(ones_f, 1.0)
            nc.vector.tensor_copy(ones1, ones_f[0:1, :])
            nc.vector.tensor_copy(ones64, ones_f[:, 0:64])
            nc.vector.tensor_copy(onesb16, ones_f[:, 0:96])
            # ones columns of [v | 1] slots
            v_ones_view = v_sb.rearrange("p s (h c) -> p s h c", c=65)[:, :, :, 64]
            nc.vector.tensor_copy(
                v_ones_view, onesb16.rearrange("p (s h) -> p s h", s=8)
            )

            qk_tiles = {}

            # ---- qk^T: one 128-feature tile (f in 0..11) ----
            def emit_qk_tile(f):
                c0 = f * 128
                psq = ps.tile([128, N], F32, tag="ps", name=f"psq_{f}")
                for d in range(D_SUB):
                    wt = wqk_pool.tile([128, 128], F32R, tag="wqk", name=f"wt_{f}_{d}")
                    nc.sync.dma_start(
                        wt, wqkv_d[d * 128 : (d + 1) * 128, c0 : c0 + 128]
                    )
                    for qh in range(2):
                        sl = slice(qh * 512, (qh + 1) * 512)
                        nc.tensor.matmul(
                            psq[:, sl],
                            lhsT=wt,
                            rhs=xT_sb[:, d, sl],
                            start=(d == 0),
                            stop=False,
                        )
                for qh in range(2):
                    sl = slice(qh * 512, (qh + 1) * 512)
                    nc.tensor.matmul(
                        psq[:, sl],
                        lhsT=bqk_sb[0:1, c0 : c0 + 128],
                        rhs=ones1,
                        start=False,
                        stop=True,
                    )
                qt = qkT_pool.tile([128, N], F32R, tag="qkT", name=f"qkT_{f}")
                nc.vector.tensor_copy(qt, psq)
                qk_tiles[f] = qt

            emit_qk_tile(0)  # q heads 0,1
            emit_qk_tile(6)  # k heads 0,1
            emit_qk_tile(1)  # q heads 2,3
            emit_qk_tile(7)  # k heads 2,3

            # ---- v m-tile: natural layout [tok, feat], scattered into
            # 65-slots; emitted inside round 0 to keep the PE dense ----
            def emit_v_tile(m):
                psv = ps.tile([128, N], F32, tag="ps", name=f"psv_{m}")
                for n0, nsz in ((0, 512), (512, 256)):
                    sl = slice(n0, n0 + nsz)
                    for d in range(D_SUB):
                        nc.tensor.matmul(
                            psv[:, sl],
                            lhsT=xT_sb[:, d, m * 128 : (m + 1) * 128],
                            rhs=wv_sb[:, d, sl],
                            start=(d == 0),
                            stop=False,
                        )
                    nc.tensor.matmul(
                        psv[:, sl],
                        lhsT=ones1[0:1, 0:128],
                        rhs=bv_sb[0:1, sl],
                        start=False,
                        stop=True,
                    )
                nc.vector.tensor_copy(
                    v_sb[:, m, :].rearrange("p (h c) -> p h c", c=65)[:, :, 0:64],
                    psv[:, 0:D].rearrange("p (h c) -> p h c", c=64),
                )

            # ---- attention rounds, software-pipelined over head pairs ----
            # round r: scores+exp for pair r (r<6), attn@v+norm for pair r-1
            attn_tiles = {}  # (pair, kt, i) -> bf16 tile
            pso_live = {}

            def emit_scores_kt(p, kt):
                qT = qk_tiles[p]
                kT = qk_tiles[6 + p]
                pss = []
                for i in range(2):
                    t = ps.tile([128, N], F32, tag="ps", name=f"pss_{p}_{kt}_{i}")
                    pss.append(t)
                # adjacent row-packed pairs: A(qh) then B(qh)
                for qh in range(2):
                    sl = slice(qh * 512, (qh + 1) * 512)
                    for i in range(2):
                        pb = slice(64 * i, 64 * i + 64)
                        nc.tensor.matmul(
                            pss[i][:, sl],
                            lhsT=kT[pb, kt * 128 : (kt + 1) * 128],
                            rhs=qT[pb, sl],
                            start=True,
                            stop=True,
                        )
                for i in range(2):
                    at = attnT_pool.tile(
                        [128, N], BF16, tag="attnT", name=f"at_{p}_{kt}_{i}"
                    )
                    nc.scalar.activation(at, pss[i], func=EXP, scale=SCALE)
                    attn_tiles[(p, kt, i)] = at

            def emit_attnv_kt(p, kt):
                for i in range(2):
                    h = 2 * p + i
                    for qh in range(2):
                        sl = slice(qh * 512, (qh + 1) * 512)
                        nc.tensor.matmul(
                            pso_live[i][0:65, sl],
                            lhsT=v_sb[:, kt, h * 65 : h * 65 + 65],
                            rhs=attn_tiles[(p, kt, i)][:, sl],
                            start=(kt == 0),
                            stop=(kt == TOK_TILES - 1),
                        )

            denrs = {}

            def emit_norm_pre(p, i):
                # DVE-only: copy den row out of PSUM, reciprocal in SBUF
                h = 2 * p + i
                dent = den_pool.tile([128, N], F32, tag="dent", name=f"dent_{h}")
                nc.vector.tensor_copy(dent[64:65, :], pso_live[i][64:65, :])
                denr = den_pool.tile([128, N], F32R, tag="denr", name=f"denr_{h}")
                with nc.allow_low_precision(
                    reason="f32r rounding of softmax denominator is in-budget"
                ):
                    nc.vector.reciprocal(denr[64:65, :], dent[64:65, :])
                denrs[i] = denr

            def emit_norm_post(p, i):
                h = 2 * p + i
                denr = denrs[i]
                psb = ps.tile([128, N], F32, tag="ps", name=f"psb_{h}")
                for qh in range(2):
                    sl = slice(qh * 512, (qh + 1) * 512)
                    nc.tensor.matmul(
                        psb[0:64, sl],
                        lhsT=ones64[64:65, :],
                        rhs=denr[64:65, sl],
                        start=True,
                        stop=True,
                    )
                psbs = den_pool.tile([64, N], F32, tag="psbs", name=f"psbs_{h}")
                nc.vector.tensor_copy(psbs, psb[0:64, :])
                nc.vector.tensor_tensor(
                    aoT_sb[64 * i : 64 * i + 64, p, :],
                    pso_live[i][0:64, :],
                    psbs,
                    MULT,
                )

            for r in range(7):
                if r >= 1:
                    pso_live = {
                        i: ps.tile(
                            [128, N], F32, tag="ps", name=f"pso_{r - 1}_{i}"
                        )
                        for i in range(2)
                    }
                for kt in range(TOK_TILES):
                    if r < 6:
                        emit_scores_kt(r, kt)
                    if r == 0:
                        emit_v_tile(kt)
                    if r >= 1:
                        emit_attnv_kt(r - 1, kt)
                if r >= 1:
                    emit_norm_pre(r - 1, 0)
                    emit_norm_pre(r - 1, 1)
                    if r + 1 < 6:
                        emit_qk_tile(r + 1)
                    emit_norm_post(r - 1, 0)
                    if r + 1 < 6:
                        emit_qk_tile(6 + r + 1)
                    emit_norm_post(r - 1, 1)

            # ---- output projection ----
            for m in range(TOK_TILES):
                psy = ps.tile([128, N], F32, tag="ps", name=f"psy_{m}")
                for n0, nsz in ((0, 512), (512, 256)):
                    sl = slice(n0, n0 + nsz)
                    for d in range(D_SUB):
                        nc.tensor.matmul(
                            psy[:, sl],
                            lhsT=aoT_sb[:, d, m * 128 : (m + 1) * 128],
                            rhs=wproj_sb[:, d, sl],
                            start=(d == 0),
                            stop=False,
                        )
                    nc.tensor.matmul(
                        psy[:, sl],
                        lhsT=ones1[0:1, 0:128],
                        rhs=bp_sb[0:1, sl],
                        start=False,
                        stop=True,
                    )
                ysb = y_pool.tile([128, D], F32, tag="ysb", name=f"ysb_{m}")
                nc.vector.tensor_copy(ysb, psy[:, 0:D])
                nc.sync.dma_start(y_d[m * 128 : (m + 1) * 128, :], ysb)

    nc.compile()
    return nc


def _in_maps(x, w_qkv, b_qkv, w_proj, b_proj):
    w_qkv = np.ascontiguousarray(w_qkv, dtype=np.float32)
    b_qkv = np.ascontiguousarray(b_qkv, dtype=np.float32)
    w_proj = np.ascontiguousarray(w_proj, dtype=np.float32)
    b_proj = np.ascontiguousarray(b_proj, dtype=np.float32)
    maps = []
    for c in range(N_CORES):
        maps.append(
            {
                "xt": np.ascontiguousarray(np.asarray(x[c], dtype=np.float32).T),
                "wqkv": w_qkv,
                "bqkv": b_qkv,
                "wproj": w_proj,
                "bproj": b_proj,
            }
        )
    return maps


def kernel(x, w_qkv, b_qkv, w_proj, b_proj):
    global _cached_nc
    if _cached_nc is None:
        _cached_nc = _build()
    from concourse.bass_utils import run_bass_kernel_spmd

    res = run_bass_kernel_spmd(
        _cached_nc,
        _in_maps(x, w_qkv, b_qkv, w_proj, b_proj),
        list(range(N_CORES)),
    )
    return np.stack([res.results[c]["y"] for c in range(N_CORES)]).astype(np.float32)


if __name__ == "__main__":
    rng = np.random.default_rng(0)
    x = rng.standard_normal((B, N, D), dtype=np.float32)
    w_qkv = rng.standard_normal((D, 3 * D), dtype=np.float32) * D**-0.5
    b_qkv = rng.standard_normal(3 * D).astype(np.float32) * 0.01
    w_proj = rng.standard_normal((D, D), dtype=np.float32) * D**-0.5
    b_proj = rng.standard_normal(D).astype(np.float32) * 0.01
    y = kernel(x, w_qkv, b_qkv, w_proj, b_proj)
    print(y.shape, y.dtype)


# revision 12
# speedup vs baseline: 1.1709x; 1.1709x over previous
"""Multi-head attention (B=8, N=1024, D=768, H=12) on 8 TRN2 NeuronCores.

Sharding: pure data parallel over batch — each core handles one batch
element; weights are replicated. No collectives.

Per-core kernel:
  1. qk^T [1536, 1024] = w_qk^T @ x^T in f32r (x fed pre-transposed from
     host); bias folded in as a K=1 matmul; result stored as fp16.
  2. per head h: scores^T [k, q] = k_h @ q_h^T in fp16 — K=64, two heads
     packed concurrently onto PE row groups 0-63 / 64-127 (fp16 is
     single-row so row groups are truly independent; f32r would burn
     both halves).  All four [128,512] outputs of a (pair, kt) step land
     in one [128, 2048] PSUM tile.
  3. softmax without max-subtraction (scores ~ N(0,1): exp overflow
     impossible): one ACT exp per (pair, kt) over the whole [128, 2048]
     PSUM tile -> fp16 attnT.
  4. attn@v in fp16: psum[0:65] += [v_h | ones]^T @ attnT — the ones
     column yields the softmax denominator in row 64 of the same PE
     stream.
  5. normalize: den row -> SBUF (PSUM-source reciprocal is broken on
     HW), gpsimd partition_broadcast to 64 rows, DVE reciprocal +
     multiply into f32r attn-out^T.
  6. proj: y = attn_out^T^T @ w_proj + bias (K=1 matmul), DMA out.

Head-pair rounds are software-pipelined: round r computes scores+exp
for pair r while the PE consumes pair r-1's attnT tiles for attn@v, so
the PE never waits on ACT mid-round.  qk^T tiles for pair r+2 are
produced at the end of round r (prefetch distance 2) so the next
round's scores can start immediately.  The v-projection fills round 0;
the output projection fills the epilogue round.
"""

import sys

sys.path.insert(0, "/opt/trn_rl_repo")

import numpy as np

B, N, D, H, HD = 8, 1024, 768, 12, 64
F_QK = 2 * D  # 1536
SCALE = HD**-0.5
TOK_TILES = N // 128  # 8
D_SUB = D // 128  # 6
N_CORES = 8

_cached_nc = None


def _build():
    import concourse.tile as tile
    from concourse import bacc, bass_isa, mybir

    F32 = mybir.dt.float32
    F32R = mybir.dt.float32r
    FP16 = mybir.dt.float16
    EXP = mybir.ActivationFunctionType.Exp
    MULT = mybir.AluOpType.mult

    nc = bacc.Bacc("TRN2", target_bir_lowering=False, debug=False)

    xt_d = nc.dram_tensor("xt", [D, N], F32R, kind="ExternalInput").ap()
    wqkv_d = nc.dram_tensor("wqkv", [D, 3 * D], F32R, kind="ExternalInput").ap()
    bqkv_d = nc.dram_tensor("bqkv", [3 * D], F32R, kind="ExternalInput").ap()
    wproj_d = nc.dram_tensor("wproj", [D, D], F32R, kind="ExternalInput").ap()
    bproj_d = nc.dram_tensor("bproj", [D], F32R, kind="ExternalInput").ap()
    y_d = nc.dram_tensor("y", [N, D], F32, kind="ExternalOutput").ap()

    with tile.TileContext(nc) as tc:
        with (
            tc.tile_pool(name="singles", bufs=1) as singles,
            tc.tile_pool(name="qkT", bufs=7) as qkT_pool,
            tc.tile_pool(name="wqk", bufs=3) as wqk_pool,
            tc.tile_pool(name="attnT", bufs=10) as attnT_pool,
            tc.tile_pool(name="den", bufs=1) as den_pool,
            tc.tile_pool(name="yout", bufs=3) as y_pool,
            tc.tile_pool(name="pso", bufs=2, space="PSUM") as ps_o,
            tc.tile_pool(name="pss", bufs=1, space="PSUM") as ps_s,
        ):
            # ---- resident SBUF tensors ----
            xT_sb = singles.tile([128, D_SUB, N], F32R)  # 24KB/part
            v_sb = singles.tile([128, TOK_TILES, H * 65], FP16)  # 12.2KB/part
            aoT_sb = singles.tile([128, D_SUB, N], F32R)  # 24KB/part
            wproj_sb = singles.tile([128, D_SUB, D], F32R)  # 18KB/part
            wv_sb = singles.tile([128, D_SUB, D], F32R)  # 18KB/part
            bqk_sb = singles.tile([1, F_QK], F32R)
            bv_sb = singles.tile([1, D], F32R)
            bp_sb = singles.tile([1, D], F32R)
            ones1 = singles.tile([1, 512], F32R)
            ones16 = singles.tile([128, 96], FP16)
            ones_f = singles.tile([128, 512], F32)

            # ---- setup (latency-critical DMAs first) ----
            xt_r = xt_d.rearrange("(o p) n -> p o n", p=128)
            for d in range(D_SUB):
                nc.sync.dma_start(xT_sb[:, d, :], xt_r[:, d, :])
            nc.sync.dma_start(bqk_sb, bqkv_d[None, 0:F_QK])
            nc.sync.dma_start(bv_sb, bqkv_d[None, F_QK : 3 * D])
            nc.sync.dma_start(bp_sb, bproj_d[None, :])
            nc.vector.memset(ones_f, 1.0)
            nc.vector.tensor_copy(ones1, ones_f[0:1, :])
            nc.vector.tensor_copy(ones16, ones_f[:, 0:96])
            # ones columns of [v | 1] slots
            v_ones_view = v_sb.rearrange("p s (h c) -> p s h c", c=65)[:, :, :, 64]
            nc.vector.tensor_copy(
                v_ones_view, ones16.rearrange("p (s h) -> p s h", s=8)
            )

            qk_tiles = {}

            # ---- qk^T: one 128-feature tile (f in 0..11), fp16 out ----
            def emit_qk_tile(f):
                c0 = f * 128
                psq = ps_o.tile([128, N], F32, tag="pso", name=f"psq_{f}")
                for d in range(D_SUB):
                    wt = wqk_pool.tile([128, 128], F32R, tag="wqk", name=f"wt_{f}_{d}")
                    nc.sync.dma_start(
                        wt, wqkv_d[d * 128 : (d + 1) * 128, c0 : c0 + 128]
                    )
                    for qh in range(2):
                        sl = slice(qh * 512, (qh + 1) * 512)
                        nc.tensor.matmul(
                            psq[:, sl],
                            lhsT=wt,
                            rhs=xT_sb[:, d, sl],
                            start=(d == 0),
                            stop=False,
                        )
                for qh in range(2):
                    sl = slice(qh * 512, (qh + 1) * 512)
                    nc.tensor.matmul(
                        psq[:, sl],
                        lhsT=bqk_sb[0:1, c0 : c0 + 128],
                        rhs=ones1,
                        start=False,
                        stop=True,
                    )
                qt = qkT_pool.tile([128, N], FP16, tag="qkT", name=f"qkT_{f}")
                nc.vector.tensor_copy(qt, psq)
                qk_tiles[f] = qt

            emit_qk_tile(0)  # q heads 0,1
            emit_qk_tile(6)  # k heads 0,1

            # bulk weight DMAs (after the first qk tiles' operands)
            nc.sync.dma_start(
                wv_sb, wqkv_d[:, F_QK:].rearrange("(o p) f -> p o f", p=128)
            )
            nc.sync.dma_start(wproj_sb, wproj_d.rearrange("(o p) f -> p o f", p=128))

            emit_qk_tile(1)  # q heads 2,3
            emit_qk_tile(7)  # k heads 2,3

            # ---- v m-tile: natural layout, scattered into 65-slots (fp16);
            # emitted inside round 0 to keep the PE dense ----
            def emit_v_tile(m):
                psv = ps_o.tile([128, N], F32, tag="pso", name=f"psv_{m}")
                for n0, nsz in ((0, 512), (512, 256)):
                    sl = slice(n0, n0 + nsz)
                    for d in range(D_SUB):
                        nc.tensor.matmul(
                            psv[:, sl],
                            lhsT=xT_sb[:, d, m * 128 : (m + 1) * 128],
                            rhs=wv_sb[:, d, sl],
                            start=(d == 0),
                            stop=False,
                        )
                    nc.tensor.matmul(
                        psv[:, sl],
                        lhsT=ones1[0:1, 0:128],
                        rhs=bv_sb[0:1, sl],
                        start=False,
                        stop=True,
                    )
                nc.vector.tensor_copy(
                    v_sb[:, m, :].rearrange("p (h c) -> p h c", c=65)[:, :, 0:64],
                    psv[:, 0:D].rearrange("p (h c) -> p h c", c=64),
                )

            # ---- attention rounds, software-pipelined over head pairs ----
            attn_tiles = {}  # (pair, kt) -> [128, 2048] fp16: [A0|B0|A1|B1]
            pso_live = {}

            def emit_scores_kt(p, kt):
                qT = qk_tiles[p]
                kT = qk_tiles[6 + p]
                pss = ps_s.tile([128, 2 * N], F32, tag="pss", name=f"pss_{p}_{kt}")
                # concurrent row-packed pairs: A(qh) at rows 0-63,
                # B(qh) at rows 64-127, adjacent in the PE stream
                for qh in range(2):
                    for i in range(2):
                        pb = slice(64 * i, 64 * i + 64)
                        sl = slice(qh * 512, (qh + 1) * 512)
                        osl = slice(qh * 1024 + i * 512, qh * 1024 + i * 512 + 512)
                        nc.tensor.matmul(
                            pss[:, osl],
                            lhsT=kT[pb, kt * 128 : (kt + 1) * 128],
                            rhs=qT[pb, sl],
                            start=True,
                            stop=True,
                        )
                at = attnT_pool.tile(
                    [128, 2 * N], FP16, tag="attnT", name=f"at_{p}_{kt}"
                )
                nc.scalar.activation(at, pss, func=EXP, scale=SCALE)
                attn_tiles[(p, kt)] = at

            def emit_attnv_kt(p, kt):
                at = attn_tiles[(p, kt)]
                for i in range(2):
                    h = 2 * p + i
                    for qh in range(2):
                        osl = slice(qh * 512, (qh + 1) * 512)
                        isl = slice(qh * 1024 + i * 512, qh * 1024 + i * 512 + 512)
                        nc.tensor.matmul(
                            pso_live[i][0:65, osl],
                            lhsT=v_sb[:, kt, h * 65 : h * 65 + 65],
                            rhs=at[:, isl],
                            start=(kt == 0),
                            stop=(kt == TOK_TILES - 1),
                        )

            def emit_norm(p, i):
                # DVE: den row PSUM->SBUF; gpsimd: broadcast to 64 rows;
                # DVE: reciprocal (64 lanes) + multiply into f32r aoT
                h = 2 * p + i
                dent = den_pool.tile([128, N], F32, tag="dent", name=f"dent_{h}")
                nc.vector.tensor_copy(dent[64:65, :], pso_live[i][64:65, :])
                denb = den_pool.tile([64, N], F32, tag="denb", name=f"denb_{h}")
                nc.gpsimd.partition_broadcast(denb, dent[64:65, :], channels=64)
                denr = den_pool.tile([64, N], F32, tag="denr", name=f"denr_{h}")
                nc.vector.reciprocal(denr, denb)
                nc.vector.tensor_tensor(
                    aoT_sb[64 * i : 64 * i + 64, p, :],
                    pso_live[i][0:64, :],
                    denr,
                    MULT,
                )

            for r in range(7):
                if r >= 1:
                    pso_live = {
                        i: ps_o.tile(
                            [128, N], F32, tag="pso", name=f"pso_{r - 1}_{i}"
                        )
                        for i in range(2)
                    }
                for kt in range(TOK_TILES):
                    if r < 6:
                        emit_scores_kt(r, kt)
                    if r == 0:
                        emit_v_tile(kt)
                    if r >= 1:
                        emit_attnv_kt(r - 1, kt)
                if r >= 1:
                    emit_norm(r - 1, 0)
                    emit_norm(r - 1, 1)
                if r + 2 < 6:
                    emit_qk_tile(r + 2)
                    emit_qk_tile(6 + r + 2)

            # ---- output projection ----
            for m in range(TOK_TILES):
                psy = ps_o.tile([128, N], F32, tag="pso", name=f"psy_{m}")
                for n0, nsz in ((0, 512), (512, 256)):
                    sl = slice(n0, n0 + nsz)
                    for d in range(D_SUB):
                        nc.tensor.matmul(
                            psy[:, sl],
                            lhsT=aoT_sb[:, d, m * 128 : (m + 1) * 128],
                            rhs=wproj_sb[:, d, sl],
                            start=(d == 0),
                            stop=False,
                        )
                    nc.tensor.matmul(
                        psy[:, sl],
                        lhsT=ones1[0:1, 0:128],
                        rhs=bp_sb[0:1, sl],
                        start=False,
                        stop=True,
                    )
                ysb = y_pool.tile([128, D], F32, tag="ysb", name=f"ysb_{m}")
                nc.vector.tensor_copy(ysb, psy[:, 0:D])
                nc.sync.dma_start(y_d[m * 128 : (m + 1) * 128, :], ysb)

    nc.compile()
    return nc


def _in_maps(x, w_qkv, b_qkv, w_proj, b_proj):
    w_qkv = np.ascontiguousarray(w_qkv, dtype=np.float32)
    b_qkv = np.ascontiguousarray(b_qkv, dtype=np.float32)
    w_proj = np.ascontiguousarray(w_proj, dtype=np.float32)
    b_proj = np.ascontiguousarray(b_proj, dtype=np.float32)
    maps = []
    for c in range(N_CORES):
        maps.append(
            {
                "xt": np.ascontiguousarray(np.asarray(x[c], dtype=np.float32).T),
                "wqkv": w_qkv,
                "bqkv": b_qkv,
                "wproj": w_proj,
                "bproj": b_proj,
            }
        )
    return maps


def kernel(x, w_qkv, b_qkv, w_proj, b_proj):
    global _cached_nc
    if _cached_nc is None:
        _cached_nc = _build()
    from concourse.bass_utils import run_bass_kernel_spmd

    res = run_bass_kernel_spmd(
        _cached_nc,
        _in_maps(x, w_qkv, b_qkv, w_proj, b_proj),
        list(range(N_CORES)),
    )
    return np.stack([res.results[c]["y"] for c in range(N_CORES)]).astype(np.float32)


if __name__ == "__main__":
    rng = np.random.default_rng(0)
    x = rng.standard_normal((B, N, D), dtype=np.float32)
    w_qkv = rng.standard_normal((D, 3 * D), dtype=np.float32) * D**-0.5
    b_qkv = rng.standard_normal(3 * D).astype(np.float32) * 0.01
    w_proj = rng.standard_normal((D, D), dtype=np.float32) * D**-0.5
    b_proj = rng.standard_normal(D).astype(np.float32) * 0.01
    y = kernel(x, w_qkv, b_qkv, w_proj, b_proj)
    print(y.shape, y.dtype)


# revision 18
# speedup vs baseline: 1.1961x; 1.0216x over previous
"""Multi-head attention (B=8, N=1024, D=768, H=12) on 8 TRN2 NeuronCores.

Sharding: pure data parallel over batch — each core handles one batch
element; weights are replicated. No collectives.

Per-core kernel:
  1. qk^T [1536, 1024] = w_qk^T @ x^T in f32r (x fed pre-transposed from
     host); bias folded in as a K=1 matmul; result stored as fp16.
  2. per head h: scores^T [k, q] = k_h @ q_h^T in fp16 — K=64, two heads
     packed concurrently onto PE row groups 0-63 / 64-127 (fp16 is
     single-row so row groups are truly independent; f32r would burn
     both halves).  All four [128,512] outputs of a (pair, kt) step land
     in one [128, 2048] PSUM tile.
  3. softmax without max-subtraction (scores ~ N(0,1): exp overflow
     impossible): one ACT exp per (pair, kt) over the whole [128, 2048]
     PSUM tile -> fp16 attnT.
  4. attn@v in fp16: psum[0:65] += [v_h | ones]^T @ attnT — the ones
     column yields the softmax denominator in row 64 of the same PE
     stream.
  5. normalize: den row -> SBUF (PSUM-source reciprocal is broken on
     HW), gpsimd partition_broadcast to 64 rows, DVE reciprocal +
     multiply into f32r attn-out^T.
  6. proj: y = attn_out^T^T @ w_proj + bias (K=1 matmul), DMA out.

Head-pair rounds are software-pipelined: round r computes scores+exp
for pair r while the PE consumes pair r-1's attnT tiles for attn@v, so
the PE never waits on ACT mid-round.  qk^T tiles for pair r+2 are
produced at the end of round r (prefetch distance 2) so the next
round's scores can start immediately.  The v-projection fills round 0;
the output projection fills the epilogue round.
"""

import sys

sys.path.insert(0, "/opt/trn_rl_repo")

import numpy as np

B, N, D, H, HD = 8, 1024, 768, 12, 64
F_QK = 2 * D  # 1536
SCALE = HD**-0.5
TOK_TILES = N // 128  # 8
D_SUB = D // 128  # 6
N_CORES = 8

_cached_nc = None


def _build():
    import concourse.tile as tile
    from concourse import bacc, bass_isa, mybir

    F32 = mybir.dt.float32
    F32R = mybir.dt.float32r
    FP16 = mybir.dt.float16
    EXP = mybir.ActivationFunctionType.Exp
    MULT = mybir.AluOpType.mult

    nc = bacc.Bacc("TRN2", target_bir_lowering=False, debug=False)

    xt_d = nc.dram_tensor("xt", [D, N], F32R, kind="ExternalInput").ap()
    wqkv_d = nc.dram_tensor("wqkv", [D, 3 * D], F32R, kind="ExternalInput").ap()
    bqkv_d = nc.dram_tensor("bqkv", [3 * D], F32R, kind="ExternalInput").ap()
    wproj_d = nc.dram_tensor("wproj", [D, D], F32R, kind="ExternalInput").ap()
    bproj_d = nc.dram_tensor("bproj", [D], F32R, kind="ExternalInput").ap()
    y_d = nc.dram_tensor("y", [N, D], F32, kind="ExternalOutput").ap()

    with tile.TileContext(nc) as tc:
        with (
            tc.tile_pool(name="singles", bufs=1) as singles,
            tc.tile_pool(name="qkT", bufs=7) as qkT_pool,
            tc.tile_pool(name="wqk", bufs=3) as wqk_pool,
            tc.tile_pool(name="attnT", bufs=10) as attnT_pool,
            tc.tile_pool(name="den", bufs=1) as den_pool,
            tc.tile_pool(name="yout", bufs=3) as y_pool,
            tc.tile_pool(name="pso", bufs=2, space="PSUM") as ps_o,
            tc.tile_pool(name="pss", bufs=1, space="PSUM") as ps_s,
            tc.tile_pool(name="dram", bufs=2, space="DRAM") as dram_pool,
        ):
            # ---- resident SBUF tensors ----
            xT_sb = singles.tile([128, D_SUB, N], F32R)  # 24KB/part
            v_sb = singles.tile([128, TOK_TILES, H * 65], FP16)  # 12.2KB/part
            aoT_sb = singles.tile([128, D_SUB, N], F32R)  # 24KB/part
            wproj_sb = singles.tile([128, D_SUB, D], F32R)  # 18KB/part
            wv_sb = singles.tile([128, D_SUB, D], F32R)  # 18KB/part
            bqk_sb = singles.tile([1, F_QK], F32R)
            bv_sb = singles.tile([1, D], F32R)
            bp_sb = singles.tile([1, D], F32R)
            ones1 = singles.tile([1, 512], F32R)
            ones64 = singles.tile([128, 64], F32R)
            ones16 = singles.tile([128, 96], FP16)
            ones_f = singles.tile([128, 512], F32)

            # ---- setup (latency-critical DMAs first) ----
            xt_r = xt_d.rearrange("(o p) n -> p o n", p=128)
            for d in range(D_SUB):
                nc.sync.dma_start(xT_sb[:, d, :], xt_r[:, d, :])
            nc.sync.dma_start(bqk_sb, bqkv_d[None, 0:F_QK])
            nc.sync.dma_start(bv_sb, bqkv_d[None, F_QK : 3 * D])
            nc.sync.dma_start(bp_sb, bproj_d[None, :])
            nc.vector.memset(ones_f, 1.0)
            nc.vector.tensor_copy(ones1, ones_f[0:1, :])
            nc.vector.tensor_copy(ones64, ones_f[:, 0:64])
            nc.vector.tensor_copy(ones16, ones_f[:, 0:96])
            # ones columns of [v | 1] slots
            v_ones_view = v_sb.rearrange("p s (h c) -> p s h c", c=65)[:, :, :, 64]
            nc.vector.tensor_copy(
                v_ones_view, ones16.rearrange("p (s h) -> p s h", s=8)
            )

            qk_tiles = {}

            # ---- qk^T: one 128-feature tile (f in 0..11), fp16 out ----
            def emit_qk_tile(f):
                c0 = f * 128
                psq = ps_o.tile([128, N], F32, tag="pso", name=f"psq_{f}")
                for d in range(D_SUB):
                    wt = wqk_pool.tile([128, 128], F32R, tag="wqk", name=f"wt_{f}_{d}")
                    nc.sync.dma_start(
                        wt, wqkv_d[d * 128 : (d + 1) * 128, c0 : c0 + 128]
                    )
                    for qh in range(2):
                        sl = slice(qh * 512, (qh + 1) * 512)
                        nc.tensor.matmul(
                            psq[:, sl],
                            lhsT=wt,
                            rhs=xT_sb[:, d, sl],
                            start=(d == 0),
                            stop=False,
                        )
                for qh in range(2):
                    sl = slice(qh * 512, (qh + 1) * 512)
                    nc.tensor.matmul(
                        psq[:, sl],
                        lhsT=bqk_sb[0:1, c0 : c0 + 128],
                        rhs=ones1,
                        start=False,
                        stop=True,
                    )
                qt = qkT_pool.tile([128, N], FP16, tag="qkT", name=f"qkT_{f}")
                nc.vector.tensor_copy(qt, psq)
                qk_tiles[f] = qt

            emit_qk_tile(0)  # q heads 0,1
            emit_qk_tile(6)  # k heads 0,1

            # bulk weight DMAs (after the first qk tiles' operands)
            nc.sync.dma_start(
                wv_sb, wqkv_d[:, F_QK:].rearrange("(o p) f -> p o f", p=128)
            )
            nc.sync.dma_start(wproj_sb, wproj_d.rearrange("(o p) f -> p o f", p=128))

            emit_qk_tile(1)  # q heads 2,3
            emit_qk_tile(7)  # k heads 2,3

            # ---- v m-tile: natural layout, scattered into 65-slots (fp16);
            # emitted inside round 0 to keep the PE dense ----
            def emit_v_tile(m):
                psv = ps_o.tile([128, N], F32, tag="pso", name=f"psv_{m}")
                for n0, nsz in ((0, 512), (512, 256)):
                    sl = slice(n0, n0 + nsz)
                    for d in range(D_SUB):
                        nc.tensor.matmul(
                            psv[:, sl],
                            lhsT=xT_sb[:, d, m * 128 : (m + 1) * 128],
                            rhs=wv_sb[:, d, sl],
                            start=(d == 0),
                            stop=False,
                        )
                    nc.tensor.matmul(
                        psv[:, sl],
                        lhsT=ones1[0:1, 0:128],
                        rhs=bv_sb[0:1, sl],
                        start=False,
                        stop=True,
                    )
                nc.vector.tensor_copy(
                    v_sb[:, m, :].rearrange("p (h c) -> p h c", c=65)[:, :, 0:64],
                    psv[:, 0:D].rearrange("p (h c) -> p h c", c=64),
                )

            # ---- attention rounds, software-pipelined over head pairs ----
            attn_tiles = {}  # (pair, kt) -> [128, 2048] fp16: [A0|B0|A1|B1]
            pso_live = {}

            def emit_scores_kt(p, kt):
                qT = qk_tiles[p]
                kT = qk_tiles[6 + p]
                pss = ps_s.tile([128, 2 * N], F32, tag="pss", name=f"pss_{p}_{kt}")
                # concurrent row-packed pairs: A(qh) at rows 0-63,
                # B(qh) at rows 64-127, adjacent in the PE stream
                for qh in range(2):
                    for i in range(2):
                        pb = slice(64 * i, 64 * i + 64)
                        sl = slice(qh * 512, (qh + 1) * 512)
                        osl = slice(qh * 1024 + i * 512, qh * 1024 + i * 512 + 512)
                        nc.tensor.matmul(
                            pss[:, osl],
                            lhsT=kT[pb, kt * 128 : (kt + 1) * 128],
                            rhs=qT[pb, sl],
                            start=True,
                            stop=True,
                        )
                at = attnT_pool.tile(
                    [128, 2 * N], FP16, tag="attnT", name=f"at_{p}_{kt}"
                )
                nc.scalar.activation(at, pss, func=EXP, scale=SCALE)
                attn_tiles[(p, kt)] = at

            def emit_attnv_kt(p, kt):
                at = attn_tiles[(p, kt)]
                for i in range(2):
                    h = 2 * p + i
                    for qh in range(2):
                        osl = slice(qh * 512, (qh + 1) * 512)
                        isl = slice(qh * 1024 + i * 512, qh * 1024 + i * 512 + 512)
                        nc.tensor.matmul(
                            pso_live[i][0:65, osl],
                            lhsT=v_sb[:, kt, h * 65 : h * 65 + 65],
                            rhs=at[:, isl],
                            start=(kt == 0),
                            stop=(kt == TOK_TILES - 1),
                        )

            def emit_norm(p, i):
                # den row PSUM->SBUF (DVE), DRAM-bounce DMA broadcast to 64
                # rows (partition-step-0 read is legal from DRAM), DVE
                # reciprocal (64 lanes) + multiply into f32r aoT
                import concourse.bass as bass

                h = 2 * p + i
                dent = den_pool.tile([128, N], F32, tag="dent", name=f"dent_{h}")
                nc.vector.tensor_copy(dent[64:65, :], pso_live[i][64:65, :])
                dend = dram_pool.tile([1, N], F32, tag="dend", name=f"dend_{h}")
                nc.sync.dma_start(dend, dent[64:65, :])
                denb = den_pool.tile([64, N], F32, tag="denb", name=f"denb_{h}")
                dend_bcast = bass.AP(
                    tensor=dend.tensor,
                    offset=dend.offset,
                    ap=[[0, 64]] + list(dend.ap[1:]),
                )
                nc.sync.dma_start(denb, dend_bcast)
                denr = den_pool.tile([64, N], F32, tag="denr", name=f"denr_{h}")
                nc.vector.reciprocal(denr, denb)
                nc.vector.tensor_tensor(
                    aoT_sb[64 * i : 64 * i + 64, p, :],
                    pso_live[i][0:64, :],
                    denr,
                    MULT,
                )

            for r in range(7):
                if r >= 1:
                    pso_live = {
                        i: ps_o.tile(
                            [128, N], F32, tag="pso", name=f"pso_{r - 1}_{i}"
                        )
                        for i in range(2)
                    }
                for kt in range(TOK_TILES):
                    if r < 6:
                        emit_scores_kt(r, kt)
                    if r == 0:
                        emit_v_tile(kt)
                    if r >= 1:
                        emit_attnv_kt(r - 1, kt)
                if r >= 1:
                    emit_norm(r - 1, 0)
                    emit_norm(r - 1, 1)
                if r + 2 < 6:
                    emit_qk_tile(r + 2)
                    emit_qk_tile(6 + r + 2)

            # ---- output projection ----
            for m in range(TOK_TILES):
                psy = ps_o.tile([128, N], F32, tag="pso", name=f"psy_{m}")
                for n0, nsz in ((0, 512), (512, 256)):
                    sl = slice(n0, n0 + nsz)
                    for d in range(D_SUB):
                        nc.tensor.matmul(
                            psy[:, sl],
                            lhsT=aoT_sb[:, d, m * 128 : (m + 1) * 128],
                            rhs=wproj_sb[:, d, sl],
                            start=(d == 0),
                            stop=False,
                        )
                    nc.tensor.matmul(
                        psy[:, sl],
                        lhsT=ones1[0:1, 0:128],
                        rhs=bp_sb[0:1, sl],
                        start=False,
                        stop=True,
                    )
                ysb = y_pool.tile([128, D], F32, tag="ysb", name=f"ysb_{m}")
                nc.vector.tensor_copy(ysb, psy[:, 0:D])
                nc.sync.dma_start(y_d[m * 128 : (m + 1) * 128, :], ysb)

    nc.compile()
    return nc


def _in_maps(x, w_qkv, b_qkv, w_proj, b_proj):
    w_qkv = np.ascontiguousarray(w_qkv, dtype=np.float32)
    b_qkv = np.ascontiguousarray(b_qkv, dtype=np.float32)
    w_proj = np.ascontiguousarray(w_proj, dtype=np.float32)
    b_proj = np.ascontiguousarray(b_proj, dtype=np.float32)
    maps = []
    for c in range(N_CORES):
        maps.append(
            {
                "xt": np.ascontiguousarray(np.asarray(x[c], dtype=np.float32).T),
                "wqkv": w_qkv,
                "bqkv": b_qkv,
                "wproj": w_proj,
                "bproj": b_proj,
            }
        )
    return maps


def kernel(x, w_qkv, b_qkv, w_proj, b_proj):
    global _cached_nc
    if _cached_nc is None:
        _cached_nc = _build()
    from concourse.bass_utils import run_bass_kernel_spmd

    res = run_bass_kernel_spmd(
        _cached_nc,
        _in_maps(x, w_qkv, b_qkv, w_proj, b_proj),
        list(range(N_CORES)),
    )
    return np.stack([res.results[c]["y"] for c in range(N_CORES)]).astype(np.float32)


if __name__ == "__main__":
    rng = np.random.default_rng(0)
    x = rng.standard_normal((B, N, D), dtype=np.float32)
    w_qkv = rng.standard_normal((D, 3 * D), dtype=np.float32) * D**-0.5
    b_qkv = rng.standard_normal(3 * D).astype(np.float32) * 0.01
    w_proj = rng.standard_normal((D, D), dtype=np.float32) * D**-0.5
    b_proj = rng.standard_normal(D).astype(np.float32) * 0.01
    y = kernel(x, w_qkv, b_qkv, w_proj, b_proj)
    print(y.shape, y.dtype)


# revision 20
# speedup vs baseline: 1.3388x; 1.1193x over previous
"""Multi-head attention (B=8, N=1024, D=768, H=12) on 8 TRN2 NeuronCores.

Sharding: pure data parallel over batch — each core handles one batch
element; weights are replicated. No collectives.

Per-core kernel:
  1. qk^T [1536, 1024] = w_qk^T @ x^T in f32r (x fed pre-transposed from
     host); bias folded in as a K=1 matmul; result stored as fp16.
  2. per head h: scores^T [k, q] = k_h @ q_h^T in fp16 — K=64, two heads
     packed concurrently onto PE row groups 0-63 / 64-127 (fp16 is
     single-row so row groups are truly independent; f32r would burn
     both halves).  All four [128,512] outputs of a (pair, kt) step land
     in one [128, 2048] PSUM tile.
  3. softmax without max-subtraction (scores ~ N(0,1): exp overflow
     impossible): one ACT exp per (pair, kt) over the whole [128, 2048]
     PSUM tile -> fp16 attnT.
  4. attn@v in fp16: psum[0:65] += [v_h | ones]^T @ attnT — the ones
     column yields the softmax denominator in row 64 of the same PE
     stream.
  5. normalize: den row -> SBUF (PSUM-source reciprocal is broken on
     HW), gpsimd partition_broadcast to 64 rows, DVE reciprocal +
     multiply into f32r attn-out^T.
  6. proj: y = attn_out^T^T @ w_proj + bias (K=1 matmul), DMA out.

Head-pair rounds are software-pipelined: round r computes scores+exp
for pair r while the PE consumes pair r-1's attnT tiles for attn@v, so
the PE never waits on ACT mid-round.  qk^T tiles for pair r+2 are
produced at the end of round r (prefetch distance 2) so the next
round's scores can start immediately.  The v-projection fills round 0;
the output projection fills the epilogue round.
"""

import sys

sys.path.insert(0, "/opt/trn_rl_repo")

import numpy as np

B, N, D, H, HD = 8, 1024, 768, 12, 64
F_QK = 2 * D  # 1536
SCALE = HD**-0.5
TOK_TILES = N // 128  # 8
D_SUB = D // 128  # 6
N_CORES = 8

_cached_nc = None


def _build():
    import concourse.tile as tile
    from concourse import bacc, bass_isa, mybir

    F32 = mybir.dt.float32
    F32R = mybir.dt.float32r
    FP16 = mybir.dt.float16
    EXP = mybir.ActivationFunctionType.Exp
    MULT = mybir.AluOpType.mult

    nc = bacc.Bacc("TRN2", target_bir_lowering=False, debug=False)

    xt_d = nc.dram_tensor("xt", [D, N], F32R, kind="ExternalInput").ap()
    wqkv_d = nc.dram_tensor("wqkv", [D, 3 * D], F32R, kind="ExternalInput").ap()
    bqkv_d = nc.dram_tensor("bqkv", [3 * D], F32R, kind="ExternalInput").ap()
    wproj_d = nc.dram_tensor("wproj", [D, D], F32R, kind="ExternalInput").ap()
    bproj_d = nc.dram_tensor("bproj", [D], F32R, kind="ExternalInput").ap()
    y_d = nc.dram_tensor("y", [N, D], F32, kind="ExternalOutput").ap()

    with tile.TileContext(nc) as tc:
        with (
            tc.tile_pool(name="singles", bufs=1) as singles,
            tc.tile_pool(name="qkT", bufs=7) as qkT_pool,
            tc.tile_pool(name="wqk", bufs=3) as wqk_pool,
            tc.tile_pool(name="attnT", bufs=10) as attnT_pool,
            tc.tile_pool(name="den", bufs=1) as den_pool,
            tc.tile_pool(name="yout", bufs=3) as y_pool,
            tc.tile_pool(name="pso", bufs=2, space="PSUM") as ps_o,
            tc.tile_pool(name="pss", bufs=1, space="PSUM") as ps_s,
            tc.tile_pool(name="dram", bufs=2, space="DRAM") as dram_pool,
        ):
            # ---- resident SBUF tensors ----
            xT_sb = singles.tile([128, D_SUB, N], F32R)  # 24KB/part
            v_sb = singles.tile([128, TOK_TILES, H * 65], FP16)  # 12.2KB/part
            aoT_sb = singles.tile([128, D_SUB, N], F32R)  # 24KB/part
            wproj_sb = singles.tile([128, D_SUB, D], F32R)  # 18KB/part
            wv_sb = singles.tile([128, D_SUB, D], F32R)  # 18KB/part
            bqk_sb = singles.tile([1, F_QK], F32R)
            bv_sb = singles.tile([1, D], F32R)
            bp_sb = singles.tile([1, D], F32R)
            ones1 = singles.tile([1, 512], F32R)
            ones64 = singles.tile([128, 64], F32R)
            ones16 = singles.tile([128, 96], FP16)
            ones_f = singles.tile([128, 512], F32)

            # ---- setup (latency-critical DMAs first) ----
            xt_r = xt_d.rearrange("(o p) n -> p o n", p=128)
            for d in range(D_SUB):
                nc.sync.dma_start(xT_sb[:, d, :], xt_r[:, d, :])
            nc.sync.dma_start(bqk_sb, bqkv_d[None, 0:F_QK])
            nc.sync.dma_start(bv_sb, bqkv_d[None, F_QK : 3 * D])
            nc.sync.dma_start(bp_sb, bproj_d[None, :])
            nc.vector.memset(ones_f, 1.0)
            nc.vector.tensor_copy(ones1, ones_f[0:1, :])
            nc.vector.tensor_copy(ones64, ones_f[:, 0:64])
            nc.vector.tensor_copy(ones16, ones_f[:, 0:96])
            # ones columns of [v | 1] slots
            v_ones_view = v_sb.rearrange("p s (h c) -> p s h c", c=65)[:, :, :, 64]
            nc.vector.tensor_copy(
                v_ones_view, ones16.rearrange("p (s h) -> p s h", s=8)
            )

            qk_tiles = {}

            # ---- qk^T: one 128-feature tile (f in 0..11), fp16 out ----
            def emit_qk_tile(f):
                c0 = f * 128
                psq = ps_o.tile([128, N], F32, tag="pso", name=f"psq_{f}")
                for d in range(D_SUB):
                    wt = wqk_pool.tile([128, 128], F32R, tag="wqk", name=f"wt_{f}_{d}")
                    nc.sync.dma_start(
                        wt, wqkv_d[d * 128 : (d + 1) * 128, c0 : c0 + 128]
                    )
                    for qh in range(2):
                        sl = slice(qh * 512, (qh + 1) * 512)
                        nc.tensor.matmul(
                            psq[:, sl],
                            lhsT=wt,
                            rhs=xT_sb[:, d, sl],
                            start=(d == 0),
                            stop=False,
                        )
                for qh in range(2):
                    sl = slice(qh * 512, (qh + 1) * 512)
                    nc.tensor.matmul(
                        psq[:, sl],
                        lhsT=bqk_sb[0:1, c0 : c0 + 128],
                        rhs=ones1,
                        start=False,
                        stop=True,
                    )
                qt = qkT_pool.tile([128, N], FP16, tag="qkT", name=f"qkT_{f}")
                nc.vector.tensor_copy(qt, psq)
                qk_tiles[f] = qt

            emit_qk_tile(0)  # q heads 0,1
            emit_qk_tile(6)  # k heads 0,1

            # bulk weight DMAs (after the first qk tiles' operands)
            nc.sync.dma_start(
                wv_sb, wqkv_d[:, F_QK:].rearrange("(o p) f -> p o f", p=128)
            )
            nc.sync.dma_start(wproj_sb, wproj_d.rearrange("(o p) f -> p o f", p=128))

            emit_qk_tile(1)  # q heads 2,3
            emit_qk_tile(7)  # k heads 2,3

            # ---- v m-tile: natural layout, scattered into 65-slots (fp16);
            # emitted inside round 0 to keep the PE dense ----
            def emit_v_tile(m):
                psv = ps_o.tile([128, N], F32, tag="pso", name=f"psv_{m}")
                for n0, nsz in ((0, 512), (512, 256)):
                    sl = slice(n0, n0 + nsz)
                    for d in range(D_SUB):
                        nc.tensor.matmul(
                            psv[:, sl],
                            lhsT=xT_sb[:, d, m * 128 : (m + 1) * 128],
                            rhs=wv_sb[:, d, sl],
                            start=(d == 0),
                            stop=False,
                        )
                    nc.tensor.matmul(
                        psv[:, sl],
                        lhsT=ones1[0:1, 0:128],
                        rhs=bv_sb[0:1, sl],
                        start=False,
                        stop=True,
                    )
                nc.vector.tensor_copy(
                    v_sb[:, m, :].rearrange("p (h c) -> p h c", c=65)[:, :, 0:64],
                    psv[:, 0:D].rearrange("p (h c) -> p h c", c=64),
                )

            # ---- attention rounds, software-pipelined over head pairs ----
            attn_tiles = {}  # (pair, kt) -> [128, 2048] fp16: [A0|B0|A1|B1]
            pso_live = {}

            def emit_scores_kt(p, kt):
                qT = qk_tiles[p]
                kT = qk_tiles[6 + p]
                pss = ps_s.tile([128, 2 * N], F32, tag="pss", name=f"pss_{p}_{kt}")
                # concurrent row-packed pairs: A(qh) at rows 0-63,
                # B(qh) at rows 64-127, adjacent in the PE stream
                for qh in range(2):
                    for i in range(2):
                        pb = slice(64 * i, 64 * i + 64)
                        sl = slice(qh * 512, (qh + 1) * 512)
                        osl = slice(qh * 1024 + i * 512, qh * 1024 + i * 512 + 512)
                        nc.tensor.matmul(
                            pss[:, osl],
                            lhsT=kT[pb, kt * 128 : (kt + 1) * 128],
                            rhs=qT[pb, sl],
                            start=True,
                            stop=True,
                        )
                at = attnT_pool.tile(
                    [128, 2 * N], FP16, tag="attnT", name=f"at_{p}_{kt}"
                )
                nc.scalar.activation(at, pss, func=EXP, scale=SCALE)
                attn_tiles[(p, kt)] = at

            def emit_attnv_kt(p, kt):
                at = attn_tiles[(p, kt)]
                for i in range(2):
                    h = 2 * p + i
                    for qh in range(2):
                        osl = slice(qh * 512, (qh + 1) * 512)
                        isl = slice(qh * 1024 + i * 512, qh * 1024 + i * 512 + 512)
                        nc.tensor.matmul(
                            pso_live[i][0:65, osl],
                            lhsT=v_sb[:, kt, h * 65 : h * 65 + 65],
                            rhs=at[:, isl],
                            start=(kt == 0),
                            stop=(kt == TOK_TILES - 1),
                        )

            def emit_norm(p, i):
                # den row PSUM->SBUF (DVE), DRAM-bounce DMA broadcast to 64
                # rows (partition-step-0 read is legal from DRAM), DVE
                # reciprocal (64 lanes) + multiply into f32r aoT
                import concourse.bass as bass

                h = 2 * p + i
                dent = den_pool.tile([128, N], F32, tag="dent", name=f"dent_{h}")
                nc.vector.tensor_copy(dent[64:65, :], pso_live[i][64:65, :])
                dend = dram_pool.tile([1, N], F32, tag="dend", name=f"dend_{h}")
                nc.sync.dma_start(dend, dent[64:65, :])
                denb = den_pool.tile([64, N], F32, tag="denb", name=f"denb_{h}")
                dend_bcast = bass.AP(
                    tensor=dend.tensor,
                    offset=dend.offset,
                    ap=[[0, 64]] + list(dend.ap[1:]),
                )
                nc.sync.dma_start(denb, dend_bcast)
                denr = den_pool.tile([64, N], F32, tag="denr", name=f"denr_{h}")
                nc.vector.reciprocal_approx_fast(out=denr, in_=denb)
                nc.vector.tensor_tensor(
                    aoT_sb[64 * i : 64 * i + 64, p, :],
                    pso_live[i][0:64, :],
                    denr,
                    MULT,
                )

            for r in range(7):
                if r >= 1:
                    pso_live = {
                        i: ps_o.tile(
                            [128, N], F32, tag="pso", name=f"pso_{r - 1}_{i}"
                        )
                        for i in range(2)
                    }
                for kt in range(TOK_TILES):
                    # attn@v first: it has no dependence on this round's
                    # ACT work, so the PE stays busy while exp(kt-1) runs
                    if r >= 1:
                        emit_attnv_kt(r - 1, kt)
                    if r < 6:
                        emit_scores_kt(r, kt)
                    if r == 0:
                        emit_v_tile(kt)
                if r >= 1:
                    emit_norm(r - 1, 0)
                    emit_norm(r - 1, 1)
                if r + 2 < 6:
                    emit_qk_tile(r + 2)
                    emit_qk_tile(6 + r + 2)

            # ---- output projection ----
            for m in range(TOK_TILES):
                psy = ps_o.tile([128, N], F32, tag="pso", name=f"psy_{m}")
                for n0, nsz in ((0, 512), (512, 256)):
                    sl = slice(n0, n0 + nsz)
                    for d in range(D_SUB):
                        nc.tensor.matmul(
                            psy[:, sl],
                            lhsT=aoT_sb[:, d, m * 128 : (m + 1) * 128],
                            rhs=wproj_sb[:, d, sl],
                            start=(d == 0),
                            stop=False,
                        )
                    nc.tensor.matmul(
                        psy[:, sl],
                        lhsT=ones1[0:1, 0:128],
                        rhs=bp_sb[0:1, sl],
                        start=False,
                        stop=True,
                    )
                ysb = y_pool.tile([128, D], F32, tag="ysb", name=f"ysb_{m}")
                nc.vector.tensor_copy(ysb, psy[:, 0:D])
                nc.sync.dma_start(y_d[m * 128 : (m + 1) * 128, :], ysb)

    nc.compile()
    return nc


def _in_maps(x, w_qkv, b_qkv, w_proj, b_proj):
    w_qkv = np.ascontiguousarray(w_qkv, dtype=np.float32)
    b_qkv = np.ascontiguousarray(b_qkv, dtype=np.float32)
    w_proj = np.ascontiguousarray(w_proj, dtype=np.float32)
    b_proj = np.ascontiguousarray(b_proj, dtype=np.float32)
    maps = []
    for c in range(N_CORES):
        maps.append(
            {
                "xt": np.ascontiguousarray(np.asarray(x[c], dtype=np.float32).T),
                "wqkv": w_qkv,
                "bqkv": b_qkv,
                "wproj": w_proj,
                "bproj": b_proj,
            }
        )
    return maps


def kernel(x, w_qkv, b_qkv, w_proj, b_proj):
    global _cached_nc
    if _cached_nc is None:
        _cached_nc = _build()
    from concourse.bass_utils import run_bass_kernel_spmd

    res = run_bass_kernel_spmd(
        _cached_nc,
        _in_maps(x, w_qkv, b_qkv, w_proj, b_proj),
        list(range(N_CORES)),
    )
    return np.stack([res.results[c]["y"] for c in range(N_CORES)]).astype(np.float32)


if __name__ == "__main__":
    rng = np.random.default_rng(0)
    x = rng.standard_normal((B, N, D), dtype=np.float32)
    w_qkv = rng.standard_normal((D, 3 * D), dtype=np.float32) * D**-0.5
    b_qkv = rng.standard_normal(3 * D).astype(np.float32) * 0.01
    w_proj = rng.standard_normal((D, D), dtype=np.float32) * D**-0.5
    b_proj = rng.standard_normal(D).astype(np.float32) * 0.01
    y = kernel(x, w_qkv, b_qkv, w_proj, b_proj)
    print(y.shape, y.dtype)
